# revision 1
# baseline (speedup 1.0000x reference)
"""Causal self-attention (B=8, T=1024, C=2048, H=16) on 8 TRN2 NeuronCores.

Strategy: data-parallel over batch — core i computes the full attention block
for batch element i (weights replicated, no collectives).

Per-core pipeline (Tile framework, all matmuls bf16 on the PE):
  A) x [T,C] f32 -> PE-transpose -> xT (bf16 would lose the f32r path; we
     transpose f32 and cast to bf16 on the PSUM->SBUF copy)
  B) qkv^T = W-chunk-stationary matmuls vs xT moving; PSUM->SBUF copies fuse
     bias (+ softmax scale for q) and cast to bf16. v-chunks are produced
     transposed and PE-transposed back to natural [T, C] layout.
  C) per head: S^T = kT-chunk^T @ qT (one matmul per 128x512 block, causally
     skipped), exp on ACT (logits are small, no max-subtraction needed),
     causal masking of diagonal blocks via precomputed 0/1 masks on DVE,
     denominators via ones-matmul (denom replicated across partitions),
     PV accumulates out^T, divide by denom on DVE -> attnT bf16.
  D) y = attnT-stationary @ w_proj (streamed, cast to bf16 on DVE),
     bias added via a K=1 ones-row matmul, output f32.
"""

import sys

if "/opt/trn_rl_repo" not in sys.path:
    sys.path.insert(0, "/opt/trn_rl_repo")

import numpy as np
import ml_dtypes

import concourse.bass as bass
import concourse.mybir as mybir
import concourse.tile as tile
from concourse import bacc
from concourse.bass_utils import run_bass_kernel_spmd

B, T, C = 8, 1024, 2048
H, HD = 16, 128
N_CORES = 8
P = 128            # partition dim
TQ = 512           # moving-operand tile (q positions per matmul)
KK = C // P        # 16 contraction tiles over C
TT = T // P        # 8 tiles over T
NQ = T // TQ       # 2 q-tiles
SCALE = 1.0 / float(np.sqrt(HD))

f32 = mybir.dt.float32
bf16 = mybir.dt.bfloat16
AFT = mybir.ActivationFunctionType

_NC_CACHE = None


def build_nc():
    nc = bacc.Bacc("TRN2", target_bir_lowering=False, debug=False,
                   num_devices=N_CORES)

    x = nc.declare_dram_parameter("x", [T, C], f32, isOutput=False)
    w_attn = nc.declare_dram_parameter("w_attn", [C, 3 * C], f32, isOutput=False)
    # b_attn pre-arranged host-side to [P, 48] (partition-major chunks,
    # q-columns pre-scaled by 1/sqrt(HD))
    b_attn = nc.declare_dram_parameter("b_attn_pm", [P, 3 * C // P], f32,
                                       isOutput=False)
    w_proj = nc.declare_dram_parameter("w_proj", [C, C], f32, isOutput=False)
    b_proj = nc.declare_dram_parameter("b_proj_row", [1, C], f32, isOutput=False)
    masks = nc.declare_dram_parameter("masks", [P, 4 * TQ], bf16, isOutput=False)
    ident_f = nc.declare_dram_parameter("ident_f", [P, P], f32, isOutput=False)
    ident_b = nc.declare_dram_parameter("ident_b", [P, P], bf16, isOutput=False)
    ones_b = nc.declare_dram_parameter("ones_b", [P, P], bf16, isOutput=False)
    y = nc.declare_dram_parameter("y", [T, C], f32, isOutput=True)

    MCH = 3 * C // P  # 48 output chunks of qkv^T

    # m-chunk processing order: v first (PV of head 0 needs all of v), then
    # (k_h, q_h) pairs so head h's S-matmuls unblock as early as possible.
    m_order = list(range(32, 48))
    for h in range(H):
        m_order.append(16 + h)
        m_order.append(h)

    with tile.TileContext(nc) as tc:
        with tc.tile_pool(name="consts", bufs=1) as consts, \
             tc.tile_pool(name="resid", bufs=1) as resid:

            # ---- constants ----
            identf_sb = consts.tile([P, P], f32, tag="identf", name="identf")
            nc.sync.dma_start(out=identf_sb, in_=ident_f[:])
            identb_sb = consts.tile([P, P], bf16, tag="identb", name="identb")
            nc.sync.dma_start(out=identb_sb, in_=ident_b[:])
            ones_sb = consts.tile([P, P], bf16, tag="ones", name="ones")
            nc.sync.dma_start(out=ones_sb, in_=ones_b[:])
            masks_sb = consts.tile([P, 4 * TQ], bf16, tag="masks", name="masks")
            nc.sync.dma_start(out=masks_sb, in_=masks[:])
            batt_sb = consts.tile([P, MCH], f32, tag="batt", name="batt")
            nc.sync.dma_start(out=batt_sb, in_=b_attn[:])
            bproj_sb = consts.tile([1, C], f32, tag="bprojf", name="bprojf")
            nc.sync.dma_start(out=bproj_sb, in_=b_proj[:])
            bproj_bf = consts.tile([1, C], bf16, tag="bprojb", name="bprojb")
            nc.vector.tensor_copy(bproj_bf, bproj_sb)

            # ---- persistent intermediates (bf16) ----
            qT = [resid.tile([P, T], bf16, tag=f"qT{i}", name=f"qT{i}") for i in range(H)]
            kT = [resid.tile([P, T], bf16, tag=f"kT{i}", name=f"kT{i}") for i in range(H)]
            v = [resid.tile([P, C], bf16, tag=f"v{i}", name=f"v{i}") for i in range(TT)]

            with tc.tile_pool(name="xT", bufs=1) as xTp, \
                 tc.tile_pool(name="wst", bufs=2) as wst, \
                 tc.tile_pool(name="vtp", bufs=2) as vtp, \
                 tc.tile_pool(name="psA", bufs=3, space=bass.MemorySpace.PSUM) as psA, \
                 tc.tile_pool(name="psB", bufs=4, space=bass.MemorySpace.PSUM) as psB:

                xT = [xTp.tile([P, T], bf16, tag=f"xT{i}", name=f"xT{i}") for i in range(KK)]

                # ---- Phase A: load x, PE-transpose into xT (f32) ----
                with tc.tile_pool(name="ldx", bufs=3) as ldx:
                    for t in range(TT):
                        x_sb = ldx.tile([P, C], f32, tag="x_sb", name="x_sb")
                        nc.sync.dma_start(out=x_sb,
                                          in_=x[t * P:(t + 1) * P, :])
                        for c in range(KK):
                            pt = psA.tile([P, P], f32, tag="pst", name="pst")
                            nc.tensor.transpose(pt, x_sb[:, c * P:(c + 1) * P],
                                                identf_sb)
                            nc.vector.tensor_copy(
                                xT[c][:, t * P:(t + 1) * P], pt)

                # ---- Phase B: qkv^T chunks ----
                w_r = w_attn[:].rearrange("(kk p) n -> p kk n", p=P)
                for m in m_order:
                    wsl = wst.tile([P, KK, P], f32, tag="wsl", name="wsl")
                    nc.sync.dma_start(out=wsl,
                                      in_=w_r[:, :, m * P:(m + 1) * P])
                    wbf = wst.tile([P, KK, P], bf16, tag="wbf", name="wbf")
                    nc.vector.tensor_copy(out=wbf, in_=wsl)

                    ps = [psB.tile([P, TQ], f32, tag="psB", name="psB") for _ in range(NQ)]
                    for kk in range(KK):
                        lhsT = wbf[:, kk, :]
                        for qt in range(NQ):
                            nc.tensor.matmul(
                                ps[qt], lhsT,
                                xT[kk][:, qt * TQ:(qt + 1) * TQ],
                                start=(kk == 0), stop=(kk == KK - 1))
                    sc = SCALE if m < 16 else 1.0
                    bias_ap = batt_sb[:, m:m + 1]
                    if m < 16:
                        dest = qT[m]
                    elif m < 32:
                        dest = kT[m - 16]
                    else:
                        dest = vtp.tile([P, T], bf16, tag="vtmp", name="vtmp")
                    for qt in range(NQ):
                        nc.scalar.activation(
                            out=dest[:, qt * TQ:(qt + 1) * TQ], in_=ps[qt],
                            func=AFT.Identity, bias=bias_ap, scale=sc)
                    if m >= 32:
                        h = m - 32
                        for kt in range(TT):
                            pv = psA.tile([P, P], bf16, tag="pst", name="pst")
                            nc.tensor.transpose(
                                pv, dest[:, kt * P:(kt + 1) * P], identb_sb)
                            nc.vector.tensor_copy(
                                v[kt][:, h * P:(h + 1) * P], pv)

            # ---- Phase C: attention per head ----
            with tc.tile_pool(name="attnp", bufs=1) as attnp:
                attnT = [attnp.tile([P, T], bf16, tag=f"attnT{i}", name=f"attnT{i}")
                         for i in range(H)]

                with tc.tile_pool(name="wpp", bufs=3) as wpp, \
                     tc.tile_pool(name="ybuf", bufs=4) as ybuf:
                  with tc.tile_pool(name="eSp", bufs=2) as eSp, \
                     tc.tile_pool(name="ctmp", bufs=2) as ctmp, \
                     tc.tile_pool(name="psS", bufs=2, space=bass.MemorySpace.PSUM) as psS, \
                     tc.tile_pool(name="psO", bufs=3, space=bass.MemorySpace.PSUM) as psO, \
                     tc.tile_pool(name="psD", bufs=3, space=bass.MemorySpace.PSUM) as psD:
                    for h in range(H):
                        eS = [eSp.tile([P, T], bf16, tag=f"eS{kt}", name=f"eS{kt}")
                              for kt in range(TT)]
                        # S^T blocks + exp (+ causal mask on diagonal blocks)
                        for kt in range(TT):
                            for qt in range(NQ):
                                if kt * P > qt * TQ + TQ - 1:
                                    continue  # fully masked
                                pss = psS.tile([P, TQ], f32, tag="psS", name="psS")
                                nc.tensor.matmul(
                                    pss, kT[h][:, kt * P:(kt + 1) * P],
                                    qT[h][:, qt * TQ:(qt + 1) * TQ],
                                    start=True, stop=True)
                                esl = eS[kt][:, qt * TQ:(qt + 1) * TQ]
                                nc.scalar.activation(out=esl, in_=pss,
                                                     func=AFT.Exp)
                                d = kt - qt * (TQ // P)
                                if 0 <= d <= 3:
                                    nc.vector.tensor_mul(
                                        esl, esl,
                                        masks_sb[:, d * TQ:(d + 1) * TQ])
                        # PV + denominators (kt-outer for weight reuse)
                        pso = [psO.tile([P, TQ], f32, tag="psO", name="psO")
                               for _ in range(NQ)]
                        psd = [psD.tile([P, TQ], f32, tag="psD", name="psD")
                               for _ in range(NQ)]
                        nkt = [(qt * (TQ // P)) + (TQ // P) for qt in range(NQ)]
                        for kt in range(TT):
                            for qt in range(NQ):
                                if kt >= nkt[qt]:
                                    continue
                                rhs = eS[kt][:, qt * TQ:(qt + 1) * TQ]
                                nc.tensor.matmul(
                                    pso[qt], v[kt][:, h * P:(h + 1) * P], rhs,
                                    start=(kt == 0), stop=(kt == nkt[qt] - 1))
                                nc.tensor.matmul(
                                    psd[qt], ones_sb, rhs,
                                    start=(kt == 0), stop=(kt == nkt[qt] - 1))
                        for qt in range(NQ):
                            rec = ctmp.tile([P, TQ], f32, tag="rec", name="rec")
                            # ~18-bit accurate, 5x faster than reciprocal();
                            # denominators are in [1, ~2e5] so edge cases are
                            # impossible
                            nc.vector.reciprocal_approx_fast(out=rec, in_=psd[qt])
                            nc.vector.tensor_mul(
                                attnT[h][:, qt * TQ:(qt + 1) * TQ],
                                pso[qt], rec)

                  # ---- Phase D: output projection ----
                  with tc.tile_pool(name="psY", bufs=8, space=bass.MemorySpace.PSUM) as psYp:
                      NCT = C // TQ  # 4
                      for ct in range(NCT):
                          psY = [psYp.tile([P, TQ], f32, tag="psY", name="psY")
                                 for _ in range(TT)]
                          for kk in range(KK):
                              wpsl = wpp.tile([P, TQ], f32, tag="wpsl", name="wpsl")
                              nc.sync.dma_start(
                                  out=wpsl,
                                  in_=w_proj[kk * P:(kk + 1) * P,
                                             ct * TQ:(ct + 1) * TQ])
                              wpbf = wpp.tile([P, TQ], bf16, tag="wpbf", name="wpbf")
                              nc.vector.tensor_copy(out=wpbf, in_=wpsl)
                              for t in range(TT):
                                  nc.tensor.matmul(
                                      psY[t], attnT[kk][:, t * P:(t + 1) * P],
                                      wpbf, start=(kk == 0), stop=False)
                          for t in range(TT):
                              # bias via K=1 ones-row matmul closing the group
                              nc.tensor.matmul(
                                  psY[t], ones_sb[0:1, :],
                                  bproj_bf[:, ct * TQ:(ct + 1) * TQ],
                                  start=False, stop=True)
                              y_sb = ybuf.tile([P, TQ], f32, tag="y_sb", name="y_sb")
                              nc.vector.tensor_copy(y_sb, psY[t])
                              nc.sync.dma_start(
                                  out=y[t * P:(t + 1) * P,
                                        ct * TQ:(ct + 1) * TQ],
                                  in_=y_sb)

    nc.compile()
    return nc


def _get_nc():
    global _NC_CACHE
    if _NC_CACHE is None:
        _NC_CACHE = build_nc()
    return _NC_CACHE


def make_in_maps(inputs):
    x = np.ascontiguousarray(np.asarray(inputs["x"], dtype=np.float32))
    w_attn = np.ascontiguousarray(np.asarray(inputs["w_attn"], dtype=np.float32))
    b_attn = np.asarray(inputs["b_attn"], dtype=np.float32)
    w_proj = np.ascontiguousarray(np.asarray(inputs["w_proj"], dtype=np.float32))
    b_proj = np.asarray(inputs["b_proj"], dtype=np.float32)

    # bias prep: [3C] -> [P, 48] partition-major; q columns folded with scale
    bpm = np.ascontiguousarray(b_attn.reshape(3 * C // P, P).T).copy()
    bpm[:, :16] *= SCALE
    bpj = np.ascontiguousarray(b_proj.reshape(1, C))

    kk_i = np.arange(P)[:, None]
    qq_i = np.arange(TQ)[None, :]
    masks = np.concatenate(
        [(qq_i >= kk_i + P * d) for d in range(4)],
        axis=1).astype(ml_dtypes.bfloat16)
    ident_f = np.eye(P, dtype=np.float32)
    ident_b = np.eye(P, dtype=ml_dtypes.bfloat16)
    ones_b = np.ones((P, P), dtype=ml_dtypes.bfloat16)

    common = dict(w_attn=w_attn, b_attn_pm=bpm, w_proj=w_proj,
                  b_proj_row=bpj, masks=masks, ident_f=ident_f,
                  ident_b=ident_b, ones_b=ones_b)
    return [dict(x=np.ascontiguousarray(x[i]), **common) for i in range(B)]


def run_spmd(inputs, trace=False, **kw):
    nc = _get_nc()
    in_maps = make_in_maps(inputs)
    return run_bass_kernel_spmd(nc, in_maps, list(range(N_CORES)),
                                trace=trace, **kw)


def kernel(**inputs):
    res = run_spmd(inputs, trace=False)
    y = np.stack([np.asarray(res.results[i]["y"]) for i in range(N_CORES)])
    return y.astype(np.float32)


if __name__ == "__main__":
    rng = np.random.default_rng(0)
    demo = {
        "x": rng.standard_normal((B, T, C)).astype(np.float32),
        "w_attn": (rng.standard_normal((C, 3 * C)) * 0.02).astype(np.float32),
        "b_attn": (rng.standard_normal(3 * C) * 0.02).astype(np.float32),
        "w_proj": (rng.standard_normal((C, C)) * 0.02).astype(np.float32),
        "b_proj": (rng.standard_normal(C) * 0.02).astype(np.float32),
    }
    out = kernel(**demo)
    print("out", out.shape, out.dtype, float(np.abs(out).max()))



# revision 2
# speedup vs baseline: 1.0190x; 1.0190x over previous
"""Causal self-attention (B=8, T=1024, C=2048, H=16) on 8 TRN2 NeuronCores.

Strategy: data-parallel over batch — core i computes the full attention block
for batch element i (weights replicated, no collectives).

v2 changes vs baseline:
  - all weights + x cast to bf16 on the HOST (no DVE cast traffic, half the
    HBM bytes); softmax scale folded into w_q/b_q host-side
  - x PE-transposed in bf16 (1 cycle/row instead of 2 for f32)
  - v computed in natural [T, C] layout directly (x-stationary matmuls
    against w_v moving) — kills 128 PE transposes + 128 DVE copies
  - w_proj bias applied by the DVE during the PSUM->SBUF copy (kills 32
    K=1 bias matmuls on the PE)

Per-core pipeline (Tile framework, all matmuls bf16 on the PE):
  A) x [T,C] bf16 -> PE-transpose -> xT
  B1) v = x @ Wv + bv in natural layout: xT-chunk stationary, Wv moving,
      8 PSUM banks (one per t-tile), DVE adds bias on PSUM->SBUF copy
  B2) q^T,k^T = W-chunk-stationary matmuls vs xT moving; PSUM->SBUF copies
      fuse bias on ACT
  C) per head: S^T = kT-chunk^T @ qT (one matmul per 128x512 block, causally
     skipped), exp on ACT, causal masking of diagonal blocks via precomputed
     0/1 masks on DVE, denominators via ones-matmul (denom replicated across
     partitions), PV accumulates out^T, divide by denom on DVE -> attnT bf16.
  D) y = attnT-stationary @ w_proj (moving, bf16), bias fused into the DVE
     PSUM->SBUF copy, output f32.
"""

import sys

if "/opt/trn_rl_repo" not in sys.path:
    sys.path.insert(0, "/opt/trn_rl_repo")

import numpy as np
import ml_dtypes

import concourse.bass as bass
import concourse.mybir as mybir
import concourse.tile as tile
from concourse import bacc
from concourse.bass_utils import run_bass_kernel_spmd

B, T, C = 8, 1024, 2048
H, HD = 16, 128
N_CORES = 8
P = 128            # partition dim
TQ = 512           # moving-operand tile (q positions per matmul)
KK = C // P        # 16 contraction tiles over C
TT = T // P        # 8 tiles over T
NQ = T // TQ       # 2 q-tiles
NCT = C // TQ      # 4 column tiles over C
SCALE = 1.0 / float(np.sqrt(HD))

f32 = mybir.dt.float32
bf16 = mybir.dt.bfloat16
AFT = mybir.ActivationFunctionType

_NC_CACHE = None


def build_nc():
    nc = bacc.Bacc("TRN2", target_bir_lowering=False, debug=False,
                   num_devices=N_CORES)

    x = nc.declare_dram_parameter("x_bf", [T, C], bf16, isOutput=False)
    # q/k weight chunks, partition-major: wqk[p, m, kk, n] =
    # w_attn[kk*128+p, m*128+n] for m < 32 (q columns pre-scaled)
    wqk = nc.declare_dram_parameter("wqk_pm", [P, 2 * KK, KK, P], bf16,
                                    isOutput=False)
    bqk = nc.declare_dram_parameter("bqk_pm", [P, 2 * KK], f32, isOutput=False)
    wv = nc.declare_dram_parameter("wv_nat", [C, C], bf16, isOutput=False)
    bv = nc.declare_dram_parameter("bv_bc", [P, C], bf16, isOutput=False)
    wp = nc.declare_dram_parameter("wp_nat", [C, C], bf16, isOutput=False)
    bp = nc.declare_dram_parameter("bp_bc", [P, C], bf16, isOutput=False)
    masks = nc.declare_dram_parameter("masks", [P, 4 * TQ], bf16, isOutput=False)
    ident_b = nc.declare_dram_parameter("ident_b", [P, P], bf16, isOutput=False)
    ones_b = nc.declare_dram_parameter("ones_b", [P, P], bf16, isOutput=False)
    y = nc.declare_dram_parameter("y", [T, C], f32, isOutput=True)

    with tile.TileContext(nc) as tc:
        with tc.tile_pool(name="consts", bufs=1) as consts, \
             tc.tile_pool(name="resid", bufs=1) as resid:

            # ---- constants ----
            identb_sb = consts.tile([P, P], bf16, tag="identb", name="identb")
            nc.sync.dma_start(out=identb_sb, in_=ident_b[:])
            ones_sb = consts.tile([P, P], bf16, tag="ones", name="ones")
            nc.sync.dma_start(out=ones_sb, in_=ones_b[:])
            masks_sb = consts.tile([P, 4 * TQ], bf16, tag="masks", name="masks")
            nc.sync.dma_start(out=masks_sb, in_=masks[:])
            bqk_sb = consts.tile([P, 2 * KK], f32, tag="bqk", name="bqk")
            nc.sync.dma_start(out=bqk_sb, in_=bqk[:])
            bv_sb = consts.tile([P, C], bf16, tag="bv", name="bv")
            nc.sync.dma_start(out=bv_sb, in_=bv[:])
            bp_sb = consts.tile([P, C], bf16, tag="bp", name="bp")
            nc.sync.dma_start(out=bp_sb, in_=bp[:])

            # ---- persistent intermediates (bf16) ----
            qT = [resid.tile([P, T], bf16, tag=f"qT{i}", name=f"qT{i}") for i in range(H)]
            kT = [resid.tile([P, T], bf16, tag=f"kT{i}", name=f"kT{i}") for i in range(H)]
            v = [resid.tile([P, C], bf16, tag=f"v{i}", name=f"v{i}") for i in range(TT)]

            with tc.tile_pool(name="xT", bufs=1) as xTp:
                xT = [xTp.tile([P, T], bf16, tag=f"xT{i}", name=f"xT{i}") for i in range(KK)]

                # ---- Phase A: load x (bf16), PE-transpose into xT ----
                with tc.tile_pool(name="ldx", bufs=3) as ldx, \
                     tc.tile_pool(name="psA", bufs=3, space=bass.MemorySpace.PSUM) as psA:
                    for t in range(TT):
                        x_sb = ldx.tile([P, C], bf16, tag="x_sb", name="x_sb")
                        eng = nc.sync if t % 2 == 0 else nc.scalar
                        eng.dma_start(out=x_sb, in_=x[t * P:(t + 1) * P, :])
                        for c in range(KK):
                            pt = psA.tile([P, P], bf16, tag="pst", name="pst")
                            nc.tensor.transpose(pt, x_sb[:, c * P:(c + 1) * P],
                                                identb_sb)
                            nc.vector.tensor_copy(
                                xT[c][:, t * P:(t + 1) * P], pt)

                # ---- Phase B1: v = x @ Wv + bv, natural layout ----
                with tc.tile_pool(name="wvp", bufs=3) as wvp, \
                     tc.tile_pool(name="psV", bufs=8, space=bass.MemorySpace.PSUM) as psVp:
                    for ct in range(NCT):
                        psV = [psVp.tile([P, TQ], f32, tag="psV", name="psV")
                               for _ in range(TT)]
                        for kk in range(KK):
                            wvt = wvp.tile([P, TQ], bf16, tag="wvt", name="wvt")
                            nc.sync.dma_start(
                                out=wvt,
                                in_=wv[kk * P:(kk + 1) * P,
                                       ct * TQ:(ct + 1) * TQ])
                            for t in range(TT):
                                nc.tensor.matmul(
                                    psV[t], xT[kk][:, t * P:(t + 1) * P], wvt,
                                    start=(kk == 0), stop=(kk == KK - 1))
                        for t in range(TT):
                            nc.vector.tensor_add(
                                v[t][:, ct * TQ:(ct + 1) * TQ], psV[t],
                                bv_sb[:, ct * TQ:(ct + 1) * TQ])

                # ---- Phase B2: q^T, k^T chunks (W stationary, xT moving) ----
                m_order = []
                for h in range(H):
                    m_order.append(KK + h)   # k chunk
                    m_order.append(h)        # q chunk
                with tc.tile_pool(name="wst", bufs=2) as wst, \
                     tc.tile_pool(name="psB", bufs=4, space=bass.MemorySpace.PSUM) as psB:
                    for m in m_order:
                        wt = wst.tile([P, KK, P], bf16, tag="wt", name="wt")
                        nc.sync.dma_start(out=wt, in_=wqk[:, m, :, :])
                        ps = [psB.tile([P, TQ], f32, tag="psB", name="psB")
                              for _ in range(NQ)]
                        for kk in range(KK):
                            lhsT = wt[:, kk, :]
                            for qt in range(NQ):
                                nc.tensor.matmul(
                                    ps[qt], lhsT,
                                    xT[kk][:, qt * TQ:(qt + 1) * TQ],
                                    start=(kk == 0), stop=(kk == KK - 1))
                        dest = qT[m] if m < KK else kT[m - KK]
                        bias_ap = bqk_sb[:, m:m + 1]
                        for qt in range(NQ):
                            nc.scalar.activation(
                                out=dest[:, qt * TQ:(qt + 1) * TQ], in_=ps[qt],
                                func=AFT.Identity, bias=bias_ap)

            # ---- Phase C: attention per head ----
            with tc.tile_pool(name="attnp", bufs=1) as attnp:
                attnT = [attnp.tile([P, T], bf16, tag=f"attnT{i}", name=f"attnT{i}")
                         for i in range(H)]

                with tc.tile_pool(name="wpp", bufs=4) as wpp, \
                     tc.tile_pool(name="ybuf", bufs=4) as ybuf:
                  with tc.tile_pool(name="eSp", bufs=2) as eSp, \
                     tc.tile_pool(name="ctmp", bufs=2) as ctmp, \
                     tc.tile_pool(name="psS", bufs=2, space=bass.MemorySpace.PSUM) as psS, \
                     tc.tile_pool(name="psO", bufs=3, space=bass.MemorySpace.PSUM) as psO, \
                     tc.tile_pool(name="psD", bufs=3, space=bass.MemorySpace.PSUM) as psD:
                    for h in range(H):
                        eS = [eSp.tile([P, T], bf16, tag=f"eS{kt}", name=f"eS{kt}")
                              for kt in range(TT)]
                        # S^T blocks + exp (+ causal mask on diagonal blocks)
                        for kt in range(TT):
                            for qt in range(NQ):
                                if kt * P > qt * TQ + TQ - 1:
                                    continue  # fully masked
                                pss = psS.tile([P, TQ], f32, tag="psS", name="psS")
                                nc.tensor.matmul(
                                    pss, kT[h][:, kt * P:(kt + 1) * P],
                                    qT[h][:, qt * TQ:(qt + 1) * TQ],
                                    start=True, stop=True)
                                esl = eS[kt][:, qt * TQ:(qt + 1) * TQ]
                                nc.scalar.activation(out=esl, in_=pss,
                                                     func=AFT.Exp)
                                d = kt - qt * (TQ // P)
                                if 0 <= d <= 3:
                                    nc.vector.tensor_mul(
                                        esl, esl,
                                        masks_sb[:, d * TQ:(d + 1) * TQ])
                        # PV + denominators (kt-outer for weight reuse)
                        pso = [psO.tile([P, TQ], f32, tag="psO", name="psO")
                               for _ in range(NQ)]
                        psd = [psD.tile([P, TQ], f32, tag="psD", name="psD")
                               for _ in range(NQ)]
                        nkt = [(qt * (TQ // P)) + (TQ // P) for qt in range(NQ)]
                        for kt in range(TT):
                            for qt in range(NQ):
                                if kt >= nkt[qt]:
                                    continue
                                rhs = eS[kt][:, qt * TQ:(qt + 1) * TQ]
                                nc.tensor.matmul(
                                    pso[qt], v[kt][:, h * P:(h + 1) * P], rhs,
                                    start=(kt == 0), stop=(kt == nkt[qt] - 1))
                                nc.tensor.matmul(
                                    psd[qt], ones_sb, rhs,
                                    start=(kt == 0), stop=(kt == nkt[qt] - 1))
                        for qt in range(NQ):
                            rec = ctmp.tile([P, TQ], f32, tag="rec", name="rec")
                            # ~18-bit accurate, 5x faster than reciprocal();
                            # denominators are in [1, ~2e5] so edge cases are
                            # impossible
                            nc.vector.reciprocal_approx_fast(out=rec, in_=psd[qt])
                            nc.vector.tensor_mul(
                                attnT[h][:, qt * TQ:(qt + 1) * TQ],
                                pso[qt], rec)

                  # ---- Phase D: output projection ----
                  with tc.tile_pool(name="psY", bufs=8, space=bass.MemorySpace.PSUM) as psYp:
                      for ct in range(NCT):
                          psY = [psYp.tile([P, TQ], f32, tag="psY", name="psY")
                                 for _ in range(TT)]
                          for kk in range(KK):
                              wpt = wpp.tile([P, TQ], bf16, tag="wpt", name="wpt")
                              nc.sync.dma_start(
                                  out=wpt,
                                  in_=wp[kk * P:(kk + 1) * P,
                                         ct * TQ:(ct + 1) * TQ])
                              for t in range(TT):
                                  nc.tensor.matmul(
                                      psY[t], attnT[kk][:, t * P:(t + 1) * P],
                                      wpt, start=(kk == 0), stop=(kk == KK - 1))
                          for t in range(TT):
                              y_sb = ybuf.tile([P, TQ], f32, tag="y_sb", name="y_sb")
                              nc.vector.tensor_add(
                                  y_sb, psY[t], bp_sb[:, ct * TQ:(ct + 1) * TQ])
                              nc.sync.dma_start(
                                  out=y[t * P:(t + 1) * P,
                                        ct * TQ:(ct + 1) * TQ],
                                  in_=y_sb)

    nc.compile()
    return nc


def _get_nc():
    global _NC_CACHE
    if _NC_CACHE is None:
        _NC_CACHE = build_nc()
    return _NC_CACHE


def make_in_maps(inputs):
    x = np.asarray(inputs["x"], dtype=np.float32)
    w_attn = np.asarray(inputs["w_attn"], dtype=np.float32)
    b_attn = np.asarray(inputs["b_attn"], dtype=np.float32)
    w_proj = np.asarray(inputs["w_proj"], dtype=np.float32)
    b_proj = np.asarray(inputs["b_proj"], dtype=np.float32)

    bf = ml_dtypes.bfloat16

    # q/k weights, scale folded into q: [P, 32, KK, P] partition-major
    wqk_f = w_attn[:, :2 * C].copy()
    wqk_f[:, :C] *= SCALE
    # [c, n] -> [kk, p, m, n'] -> [p, m, kk, n']
    wqk_pm = np.ascontiguousarray(
        wqk_f.reshape(KK, P, 2 * KK, P).transpose(1, 2, 0, 3)).astype(bf)

    bqk_f = b_attn[:2 * C].copy()
    bqk_f[:C] *= SCALE
    bqk_pm = np.ascontiguousarray(bqk_f.reshape(2 * KK, P).T).astype(np.float32)

    wv_nat = np.ascontiguousarray(w_attn[:, 2 * C:]).astype(bf)
    bv_bc = np.ascontiguousarray(
        np.broadcast_to(b_attn[2 * C:], (P, C))).astype(bf)

    wp_nat = np.ascontiguousarray(w_proj).astype(bf)
    bp_bc = np.ascontiguousarray(np.broadcast_to(b_proj, (P, C))).astype(bf)

    kk_i = np.arange(P)[:, None]
    qq_i = np.arange(TQ)[None, :]
    masks = np.concatenate(
        [(qq_i >= kk_i + P * d) for d in range(4)],
        axis=1).astype(bf)
    ident_b = np.eye(P, dtype=bf)
    ones_b = np.ones((P, P), dtype=bf)

    common = dict(wqk_pm=wqk_pm, bqk_pm=bqk_pm, wv_nat=wv_nat, bv_bc=bv_bc,
                  wp_nat=wp_nat, bp_bc=bp_bc, masks=masks, ident_b=ident_b,
                  ones_b=ones_b)
    x_bf = np.ascontiguousarray(x).astype(bf)
    return [dict(x_bf=np.ascontiguousarray(x_bf[i]), **common)
            for i in range(B)]


def run_spmd(inputs, trace=False, **kw):
    nc = _get_nc()
    in_maps = make_in_maps(inputs)
    return run_bass_kernel_spmd(nc, in_maps, list(range(N_CORES)),
                                trace=trace, **kw)


def kernel(**inputs):
    res = run_spmd(inputs, trace=False)
    y = np.stack([np.asarray(res.results[i]["y"]) for i in range(N_CORES)])
    return y.astype(np.float32)


if __name__ == "__main__":
    rng = np.random.default_rng(0)
    demo = {
        "x": rng.standard_normal((B, T, C)).astype(np.float32),
        "w_attn": (rng.standard_normal((C, 3 * C)) * 0.02).astype(np.float32),
        "b_attn": (rng.standard_normal(3 * C) * 0.02).astype(np.float32),
        "w_proj": (rng.standard_normal((C, C)) * 0.02).astype(np.float32),
        "b_proj": (rng.standard_normal(C) * 0.02).astype(np.float32),
    }
    out = kernel(**demo)
    print("out", out.shape, out.dtype, float(np.abs(out).max()))
